# revision 17
# baseline (speedup 1.0000x reference)
"""Trainium2 Bass kernel for nn_CrossNetworkMix (cross-network MoE mixer).

Reference computation (per layer, L=3):
    gate = softmax(xl @ gate_w.T)                 # [B, E]
    Vx   = tanh(einsum('edr,bd->ber', V[l], xl))  # [B, E, R]
    CVx  = tanh(einsum('esr,bes->ber', C[l], Vx)) # [B, E, R]
    UCVx = einsum('edr,ber->bed', U[l], CVx)      # [B, E, D]
    xl   = einsum('be,bed->bd', gate, x0[:,None,:]*(UCVx + bias[l])) + xl

Key algebraic identity used: sum_e gate[b,e] = 1, so
    xl_{l+1} = x0 dot (mix_l + bias_l) + xl_l,  mix_l = sum_e g_e * (U_e @ tanh(...))
and by induction xl_l = x0 * A_l with A_l = 1 + sum_{j<l} (mix_j + bias_j).
The kernel carries the accumulator A (residual never materialized in f32).

Device layout: activations transposed [feature, batch]; batch B=16384 sharded
8 ways (2048 cols/core), processed in 4 column tiles of 512 (PSUM free dim).
Experts packed in pairs into 128-wide matmuls. All matmuls bf16 with f32 PSUM
accumulation; softmax reduction/broadcast across the tiny E=4 partition dim is
done with small matmuls (ones / selector / identity stationary operands).

Scheduling notes (all measured on HW traces):
 - The PE clock ramps 0.65 -> 1.2 -> 2.4 GHz and needs ~3us of continuous
   work for full speed, so dummy warm-up matmuls run while the first DMAs
   land.
 - x is loaded in per-batch-tile pieces so tile 0's matmuls start ~1.5us in.
 - Elementwise xl = A*x0 products run on the otherwise-idle Pool (GpSimd)
   engine.  DVE ops are arranged to have at most one SBUF read operand so
   they never arbitrate for the DVE/Pool shared SBUF port (exclusive lock).
 - Weight loads go out on the Act HWDGE queue, x loads + output stores on
   the SP queue, so neither queue's ~0.6us per-DMA issue time serializes.
"""

from contextlib import ExitStack

import numpy as np
import ml_dtypes

import concourse.bass as bass
import concourse.tile as tile
from concourse import bacc
from concourse import mybir
from concourse.bass_utils import run_bass_kernel_spmd

BF = ml_dtypes.bfloat16

B, D, L, R, E = 16384, 512, 3, 64, 4
NCORES = 8
BLOC = B // NCORES          # batch columns per core
NT = BLOC // 512            # batch tiles per core (512 cols each)
BT = 512                    # batch tile width
NC_CH = D // 128            # feature chunks (4)
NG = E // 2                 # expert pair groups (2)
NWARM = 8                   # PE warm-up matmuls (p-state ramp + DMA shadow)

_cache = {}


def _build():
    nc = bacc.Bacc("TRN2", target_bir_lowering=False, debug=False)
    dt = mybir.dt

    # ---- DRAM I/O ----
    # x per batch tile, chunk-major in the free dim: [t][p][c*BT+j] = x[c*128+p, t*BT+j]
    xbt = nc.dram_tensor("xbt", [NT, 128, NC_CH * BT], dt.bfloat16, kind="ExternalInput")
    xft = nc.dram_tensor("xft", [NT, 128, NC_CH * BT], dt.float32, kind="ExternalInput")
    # weights packed per layer into one blob: vg (8x128 cols) | cbd (2x128) |
    # ust (2x512) = [128, 2304]; consts into one bf16 blob = [128, 400]
    wblob = nc.dram_tensor("wblob", [L, 128, 2304], dt.bfloat16, kind="ExternalInput")
    cblob = nc.dram_tensor("cblob", [128, 400], dt.bfloat16, kind="ExternalInput")
    biasp = nc.dram_tensor("biasp", [128, L * NC_CH], dt.float32, kind="ExternalInput")
    outT = nc.dram_tensor("outT", [NC_CH, 128, BLOC], dt.float32, kind="ExternalOutput")

    with tile.TileContext(nc) as tc, ExitStack() as ctx:
        pers = ctx.enter_context(tc.tile_pool(name="pers", bufs=1))
        psg = ctx.enter_context(tc.tile_pool(name="psg", bufs=1, space="PSUM"))
        psgb = ctx.enter_context(tc.tile_pool(name="psgb", bufs=1, space="PSUM"))
        psex = ctx.enter_context(tc.tile_pool(name="psex", bufs=2, space="PSUM"))
        psa = ctx.enter_context(tc.tile_pool(name="psa", bufs=2, space="PSUM"))
        sm = ctx.enter_context(tc.tile_pool(name="sm", bufs=4))     # small gate tiles
        work = ctx.enter_context(tc.tile_pool(name="work", bufs=3))  # tanh/gc tiles
        xla = ctx.enter_context(tc.tile_pool(name="xla", bufs=NT * NG + 2))   # xl pair tiles
        apool = ctx.enter_context(tc.tile_pool(name="apool", bufs=NT * NG + 2))  # A pair tiles
        opool = ctx.enter_context(tc.tile_pool(name="opool", bufs=NT * NC_CH))

        # ---- PE warm-up: ramp the p-state clock while the first DMAs land.
        # warm tile is memset by Pool (no DMA dependency) so the PE can start
        # within ~1us of kernel entry; each matmul streams 512 cols.
        warm = pers.tile([128, BT], dt.bfloat16, name="warm", tag="warm")
        nc.gpsimd.memset(warm[:], 0)
        # warm-up matmuls write the same bank the gate logits use later
        small_ps = psg.tile([128, BT], dt.float32, name="small_ps", tag="small_ps")
        for i in range(NWARM):
            nc.tensor.matmul(small_ps[:], warm[:, 0:128], warm[:],
                             start=True, stop=True, skip_group_check=True)

        # ---- persistent loads ----
        # Two HWDGE queues: weights (cblob/wblob/bias) on the Act queue,
        # x tiles + stores on the SP queue, issued in consumption order.
        cb_sb = pers.tile([128, 400], dt.bfloat16, name="cb_sb", tag="cb_sb")
        nc.scalar.dma_start(cb_sb[:], cblob.ap())
        gwt_sb = cb_sb[:, 0:16]
        eye_sb = cb_sb[:, 16:144]
        sel_sb = cb_sb[0:4, 144:400]

        # bias is tiny and needed by layer-0 phase 3 (~15us in): load it
        # before the big wblob transfers so x-load bandwidth contention
        # can't push it past its first use (measured: a late bias stalls
        # the whole statically-scheduled ACT stream ~10us).
        bias_sb = pers.tile([128, L * NC_CH], dt.float32, name="bias_sb", tag="bias_sb")
        nc.scalar.dma_start(bias_sb[:], biasp.ap())

        x0b = []
        for t in range(NT):
            xt = pers.tile([128, NC_CH * BT], dt.bfloat16, name=f"x0b{t}", tag=f"x0b{t}")
            nc.sync.dma_start(xt[:], xbt.ap()[t])
            x0b.append(xt)

        wl_sb = []
        vg_sb = {}
        cbd_sb = {}
        ust_sb = {}
        for l in range(L):
            t_ = pers.tile([128, 2304], dt.bfloat16, name=f"wl{l}", tag=f"wl{l}")
            nc.scalar.dma_start(t_[:], wblob.ap()[l])
            wl_sb.append(t_)
            for g in range(NG):
                for c in range(NC_CH):
                    vg_sb[(l, g, c)] = t_[:, 128 * (4 * g + c):128 * (4 * g + c + 1)]
                cbd_sb[(l, g)] = t_[:, 1024 + 128 * g:1024 + 128 * (g + 1)]
                ust_sb[(l, g)] = t_[:, 1280 + 512 * g:1280 + 512 * (g + 1)]

        # f32 x (final-layer epilogue only, ~70us in) rides the Act ring
        # BEHIND the weight blobs: descriptors are FIFO per DMA engine, so
        # issuing these 4MB first would push wl0 past its first use.
        x0f = []
        for t in range(NT):
            xt = pers.tile([128, NC_CH * BT], dt.float32, name=f"x0f{t}", tag=f"x0f{t}")
            nc.scalar.dma_start(xt[:], xft.ap()[t])
            x0f.append(xt)

        # Single-wait discipline: DVE/ACT/Pool instructions carry at most one
        # sem wait in this ISA. Touch DMA-loaded regions just before the
        # engine first reads them so each compute op has one un-observed
        # dependency and no toucher stalls on a DMA that lands late.
        scratch = pers.tile([4, 16], dt.float32, name="scratch", tag="scratch")

        Exp = mybir.ActivationFunctionType.Exp
        Tanh = mybir.ActivationFunctionType.Tanh
        Ident = mybir.ActivationFunctionType.Identity
        add = mybir.AluOpType.add
        mult = mybir.AluOpType.mult

        A_sb = {}     # (t, g) -> SBUF bf16 A pair tile [128, 2*BT] of current layer
        xl_sb = {}    # (t, g) -> SBUF bf16 xl pair tile [128, 2*BT]

        def xsrc(l, c, t):
            if l == 0:
                return x0b[t][:, bass.ts(c, BT)]
            return xl_sb[(t, c // 2)][:, bass.ts(c % 2, BT)]

        for l in range(L):
            A_new = {}
            xl_new = {}

            # ---- phase 1: gate logits + softmax -> g_all[t] (bf16 SBUF) ----
            g_all = {}
            for t in range(NT):
                # logits for tile t at partitions 32*(t%3): psum base
                # partition must be 0/32/64; tile 3 reuses tile 0's rows
                # (its exp read is long done by then)
                pb = 32 * (t % 3)
                lg = small_ps[pb:pb + 4, :]
                for c in range(NC_CH):
                    nc.tensor.matmul(
                        lg, gwt_sb[:, bass.ts(c, 4)], xsrc(l, c, t),
                        start=(c == 0), stop=(c == NC_CH - 1),
                    )
                # Softmax normalization with batch on partitions: 32x32
                # stream-transpose puts the 4 expert weights of 32 batch cols
                # in each partition row; reduce+reciprocal then run on 128
                # lanes instead of 4 (a [4,BT] reciprocal costs ~3.3us).
                w_sb = sm.tile([32, BT], dt.bfloat16, name=f"w{l}{t}", tag="w")
                nc.scalar.activation(w_sb[0:4, :], lg, Exp)
                wt = sm.tile([32, BT], dt.bfloat16, name=f"wt{l}{t}", tag="wt")
                nc.vector.transpose(wt[:], w_sb[:])
                wt3 = wt[:].rearrange("p (b q) -> p b q", q=32)[:, :, 0:4]
                s32 = sm.tile([32, 16], dt.float32, name=f"s32{l}{t}", tag="s32")
                nc.vector.tensor_reduce(s32[:], wt3, axis=mybir.AxisListType.X,
                                        op=mybir.AluOpType.add)
                rs32 = sm.tile([32, 16], dt.float32, name=f"rs32{l}{t}", tag="rs32")
                nc.vector.reciprocal(rs32[:], s32[:])
                gt = sm.tile([32, BT], dt.bfloat16, name=f"gt{l}{t}", tag="gt")
                gt3 = gt[:].rearrange("p (b q) -> p b q", q=32)[:, :, 0:4]
                rs3 = rs32[:].rearrange("p b -> p b ()").broadcast_to([32, 16, 4])
                # normalize on Pool: keeps both-SBUF-operand multiplies off
                # DVE's shared port (Pool would block them for a whole op)
                nc.gpsimd.tensor_tensor(gt3, wt3, rs3, op=mybir.AluOpType.mult)
                g_sb = sm.tile([32, BT], dt.bfloat16, name=f"g{l}{t}", tag="g")
                nc.vector.transpose(g_sb[:], gt[:])
                g_all[t] = g_sb

            # ---- phase 2: experts (V -> tanh -> C -> tanh -> gate) ----
            gc_all = {}

            def emit_v(t):
                # both expert pairs share wide [128,1024] psum tiles so each
                # tanh is one ACT op instead of two
                vx = psex.tile([128, 2 * BT], dt.float32, name=f"vx{l}{t}", tag="ex")
                for g in range(NG):
                    for c in range(NC_CH):
                        nc.tensor.matmul(vx[:, bass.ts(g, BT)], vg_sb[(l, g, c)],
                                         xsrc(l, c, t),
                                         start=(c == 0), stop=(c == NC_CH - 1))
                tv = work.tile([128, 2 * BT], dt.bfloat16, name=f"tv{l}{t}", tag="tv")
                nc.scalar.activation(tv[:], vx[:], Tanh)
                return tv

            def emit_c(t, tv):
                cv = psex.tile([128, 2 * BT], dt.float32, name=f"cv{l}{t}", tag="ex")
                for g in range(NG):
                    nc.tensor.matmul(cv[:, bass.ts(g, BT)], cbd_sb[(l, g)],
                                     tv[:, bass.ts(g, BT)], start=True, stop=True)
                tcv = work.tile([128, 2 * BT], dt.bfloat16, name=f"tc{l}{t}", tag="tc")
                nc.scalar.activation(tcv[:], cv[:], Tanh)
                gc_sb = []
                for g in range(NG):
                    # gate broadcast just-in-time; gb has one bank, so probe
                    # (advancing DVE's PE clock) precedes each gating mul
                    p = psgb.tile([128, BT], dt.float32, name=f"gb{l}{t}{g}", tag="gb")
                    nc.tensor.matmul(p[:], sel_sb[:, bass.ts(g, 128)], g_all[t][0:4, :],
                                     start=True, stop=True)
                    nc.vector.tensor_copy(scratch[:, 8:12], p[0:4, 0:4])
                    gc = work.tile([128, BT], dt.bfloat16, name=f"gc{l}{t}{g}", tag="gc")
                    nc.vector.tensor_mul(gc[:], tcv[:, bass.ts(g, BT)], p[:])
                    gc_sb.append(gc)
                gc_all[t] = gc_sb

            # one-tile lookahead: V(t+1) is emitted before C(t) so the PE
            # stays busy across the tanh latency of tile t
            tv_pend = {}
            tv_pend[0] = emit_v(0)
            for t in range(NT):
                if t + 1 < NT:
                    tv_pend[t + 1] = emit_v(t + 1)
                emit_c(t, tv_pend.pop(t))

            # ---- phase 3: U matmuls + residual accumulate + epilogue ----
            def phase3(t):
                gc_sb = gc_all.pop(t)
                if l == L - 1:
                    nc.vector.tensor_copy(scratch[:, 0:4], x0f[t][0:4, 0:4])
                ap_pend = {}
                for g in range(NG):
                    an = apool.tile([128, 2 * BT], dt.bfloat16, name=f"A{l}{t}{g}", tag="A")
                    A_new[(t, g)] = an
                for c in range(NC_CH):
                    pa = psa.tile([128, BT], dt.float32, name=f"pa{l}{t}{c}", tag="pa")
                    nc.tensor.matmul(pa[:], ust_sb[(l, 0)][:, bass.ts(c, 128)],
                                     gc_sb[0][:], start=True, stop=False)
                    nc.tensor.matmul(pa[:], ust_sb[(l, 1)][:, bass.ts(c, 128)],
                                     gc_sb[1][:], start=False, stop=(l != 2))
                    if l == 2:
                        nc.tensor.matmul(pa[:], eye_sb,
                                         A_sb[(t, c // 2)][:, bass.ts(c % 2, BT)],
                                         start=False, stop=True)
                    bias_ap = bias_sb[:, l * NC_CH + c: l * NC_CH + c + 1]
                    if l == 0:
                        an = A_new[(t, c // 2)]
                        nc.scalar.activation(an[:, bass.ts(c % 2, BT)], pa[:],
                                             Ident, bias=bias_ap)
                    elif l == 1:
                        # A' = (mix + bias) + A_prev in one DVE op (psum in0 +
                        # single SBUF read keeps the shared port free)
                        an = A_new[(t, c // 2)]
                        nc.vector.scalar_tensor_tensor(
                            out=an[:, bass.ts(c % 2, BT)], in0=pa[:], scalar=bias_ap,
                            in1=A_sb[(t, c // 2)][:, bass.ts(c % 2, BT)],
                            op0=add, op1=add,
                        )
                    else:
                        # stores alternate across both HWDGE rings so the
                        # final 4MB drain isn't serialized on one issue queue
                        ot = opool.tile([128, BT], dt.float32, name=f"o{l}{t}{c}", tag="o")
                        nc.vector.scalar_tensor_tensor(
                            out=ot[:], in0=pa[:], scalar=bias_ap,
                            in1=x0f[t][:, bass.ts(c, BT)], op0=add, op1=mult,
                        )
                        eng = nc.sync if c % 2 == 0 else nc.scalar
                        eng.dma_start(outT.ap()[c, :, t * BT:(t + 1) * BT], ot[:])
                if l < L - 1:
                    # xl = A * x0 on the Pool engine (its own SBUF port;
                    # 2-SBUF-operand multiplies would contend on DVE)
                    for g in range(NG):
                        xn = xla.tile([128, 2 * BT], dt.bfloat16, name=f"xl{l}{t}{g}", tag="xl")
                        nc.gpsimd.tensor_mul(xn[:], A_new[(t, g)][:],
                                             x0b[t][:, bass.ts(g, 2 * BT)])
                        xl_new[(t, g)] = xn

            if l == 0:
                nc.scalar.copy(scratch[:, 4:8], bias_sb[0:4, 0:4])
            for t in range(NT):
                phase3(t)
            A_sb = A_new
            xl_sb = xl_new

    nc.compile()
    return nc


def _pack_inputs(input, U, V, C, bias, gate_w):
    """Host-side packing of full inputs into per-core DRAM tensor maps."""
    x = np.asarray(input, dtype=np.float32)
    U = np.asarray(U, dtype=np.float32)
    V = np.asarray(V, dtype=np.float32)
    C = np.asarray(C, dtype=np.float32)
    bias = np.asarray(bias, dtype=np.float32)
    gate_w = np.asarray(gate_w, dtype=np.float32)

    xT = np.ascontiguousarray(x.T)                      # [D, B]
    wblob = np.zeros((L, 128, 2304), dtype=BF)
    for l in range(L):
        for g in range(NG):
            pair = np.concatenate([V[l, 2 * g], V[l, 2 * g + 1]], axis=1)  # [512,128]
            pr = pair.reshape(NC_CH, 128, 128)
            for c in range(NC_CH):
                wblob[l, :, 128 * (4 * g + c):128 * (4 * g + c + 1)] = pr[c].astype(BF)
            cb = np.zeros((128, 128), dtype=np.float32)
            cb[0:64, 0:64] = C[l, 2 * g]
            cb[64:128, 64:128] = C[l, 2 * g + 1]
            wblob[l, :, 1024 + 128 * g:1024 + 128 * (g + 1)] = cb.astype(BF)
            wblob[l, :, 1280 + 512 * g:1280 + 512 * (g + 1)] = np.concatenate(
                [U[l, 2 * g].T, U[l, 2 * g + 1].T], axis=0).astype(BF)
    cblob = np.zeros((128, 400), dtype=BF)
    cblob[:, 0:16] = gate_w.reshape(E, NC_CH, 128).transpose(2, 1, 0).reshape(128, NC_CH * E).astype(BF)
    cblob[:, 16:144] = np.eye(128, dtype=BF)
    for g in range(NG):
        cblob[2 * g, 144 + 128 * g:144 + 128 * g + 64] = 1
        cblob[2 * g + 1, 144 + 128 * g + 64:144 + 128 * (g + 1)] = 1
    biasp = np.empty((128, L * NC_CH), dtype=np.float32)
    for l in range(L):
        for c in range(NC_CH):
            biasp[:, l * NC_CH + c] = bias[l, 128 * c: 128 * (c + 1)]
    biasp[:, 0:NC_CH] += 1.0    # layer-0 folds A=1 init

    shared = {
        "wblob": wblob,
        "cblob": cblob,
        "biasp": biasp,
    }
    in_maps = []
    for k in range(NCORES):
        xs = xT[:, k * BLOC:(k + 1) * BLOC]             # [512, 2048]
        # per batch tile, chunk-major free dim: [t, p, c*BT+j] = xs[c*128+p, t*BT+j]
        xtile = np.ascontiguousarray(
            xs.reshape(NC_CH, 128, NT, BT).transpose(2, 1, 0, 3).reshape(NT, 128, NC_CH * BT)
        )
        m = dict(shared)
        m["xbt"] = xtile.astype(BF)
        m["xft"] = xtile
        in_maps.append(m)
    return in_maps


def run(inputs, trace=False):
    if "nc" not in _cache:
        _cache["nc"] = _build()
    nc = _cache["nc"]
    in_maps = _pack_inputs(**inputs)
    res = run_bass_kernel_spmd(nc, in_maps, list(range(NCORES)), trace=trace)
    outs = []
    for k in range(NCORES):
        oT = np.asarray(res.results[k]["outT"], dtype=np.float32)  # [4,128,2048]
        outs.append(oT.reshape(D, BLOC).T)                         # [2048, 512]
    full = np.concatenate(outs, axis=0)                            # [B, D]
    return full, res


def kernel(**inputs):
    return run(inputs)[0]


# revision 19
# speedup vs baseline: 1.1421x; 1.1421x over previous
"""Trainium2 Bass kernel for nn_CrossNetworkMix (cross-network MoE mixer).

Reference computation (per layer, L=3):
    gate = softmax(xl @ gate_w.T)                 # [B, E]
    Vx   = tanh(einsum('edr,bd->ber', V[l], xl))  # [B, E, R]
    CVx  = tanh(einsum('esr,bes->ber', C[l], Vx)) # [B, E, R]
    UCVx = einsum('edr,ber->bed', U[l], CVx)      # [B, E, D]
    xl   = einsum('be,bed->bd', gate, x0[:,None,:]*(UCVx + bias[l])) + xl

Key algebraic identity used: sum_e gate[b,e] = 1, so
    xl_{l+1} = x0 dot (mix_l + bias_l) + xl_l,  mix_l = sum_e g_e * (U_e @ tanh(...))
and by induction xl_l = x0 * A_l with A_l = 1 + sum_{j<l} (mix_j + bias_j).
The kernel carries the accumulator A (residual never materialized in f32).

Device layout: activations transposed [feature, batch]; batch B=16384 sharded
8 ways (2048 cols/core), processed in 4 column tiles of 512 (PSUM free dim).
Experts packed in pairs into 128-wide matmuls. All matmuls bf16 with f32 PSUM
accumulation; softmax reduction/broadcast across the tiny E=4 partition dim is
done with small matmuls (ones / selector / identity stationary operands).

Scheduling notes (all measured on HW traces):
 - The PE clock ramps 0.65 -> 1.2 -> 2.4 GHz and needs ~3us of continuous
   work for full speed, so dummy warm-up matmuls run while the first DMAs
   land.
 - x is loaded in per-batch-tile pieces so tile 0's matmuls start ~1.5us in.
 - Elementwise xl = A*x0 products run on the otherwise-idle Pool (GpSimd)
   engine.  DVE ops are arranged to have at most one SBUF read operand so
   they never arbitrate for the DVE/Pool shared SBUF port (exclusive lock).
 - Weight loads go out on the Act HWDGE queue, x loads + output stores on
   the SP queue, so neither queue's ~0.6us per-DMA issue time serializes.
"""

from contextlib import ExitStack

import numpy as np
import ml_dtypes

import concourse.bass as bass
import concourse.tile as tile
from concourse import bacc
from concourse import mybir
from concourse.bass_utils import run_bass_kernel_spmd

BF = ml_dtypes.bfloat16

B, D, L, R, E = 16384, 512, 3, 64, 4
NCORES = 8
BLOC = B // NCORES          # batch columns per core
NT = BLOC // 512            # batch tiles per core (512 cols each)
BT = 512                    # batch tile width
NC_CH = D // 128            # feature chunks (4)
NG = E // 2                 # expert pair groups (2)
NWARM = 8                   # PE warm-up matmuls (p-state ramp + DMA shadow)

_cache = {}


def _build():
    nc = bacc.Bacc("TRN2", target_bir_lowering=False, debug=False)
    dt = mybir.dt

    # ---- DRAM I/O ----
    # x per batch tile, chunk-major in the free dim: [t][p][c*BT+j] = x[c*128+p, t*BT+j]
    xbt = nc.dram_tensor("xbt", [NT, 128, NC_CH * BT], dt.bfloat16, kind="ExternalInput")
    xft = nc.dram_tensor("xft", [NT, 128, NC_CH * BT], dt.float32, kind="ExternalInput")
    # weights packed per layer into one blob: vg (8x128 cols) | cbd (2x128) |
    # ust (2x512) = [128, 2304]; consts into one bf16 blob = [128, 400]
    wblob = nc.dram_tensor("wblob", [L, 128, 2304], dt.bfloat16, kind="ExternalInput")
    cblob = nc.dram_tensor("cblob", [128, 400], dt.bfloat16, kind="ExternalInput")
    biasp = nc.dram_tensor("biasp", [128, L * NC_CH], dt.float32, kind="ExternalInput")
    outT = nc.dram_tensor("outT", [NC_CH, 128, BLOC], dt.float32, kind="ExternalOutput")

    with tile.TileContext(nc) as tc, ExitStack() as ctx:
        pers = ctx.enter_context(tc.tile_pool(name="pers", bufs=1))
        psg = ctx.enter_context(tc.tile_pool(name="psg", bufs=1, space="PSUM"))
        psgb = ctx.enter_context(tc.tile_pool(name="psgb", bufs=1, space="PSUM"))
        psex = ctx.enter_context(tc.tile_pool(name="psex", bufs=2, space="PSUM"))
        psa = ctx.enter_context(tc.tile_pool(name="psa", bufs=2, space="PSUM"))
        sm = ctx.enter_context(tc.tile_pool(name="sm", bufs=4))     # small gate tiles
        work = ctx.enter_context(tc.tile_pool(name="work", bufs=3))  # tanh/gc tiles
        xla = ctx.enter_context(tc.tile_pool(name="xla", bufs=NT * NG + 2))   # xl pair tiles
        apool = ctx.enter_context(tc.tile_pool(name="apool", bufs=NT * NG + 2))  # A pair tiles
        opool = ctx.enter_context(tc.tile_pool(name="opool", bufs=NT * NC_CH))

        # ---- PE warm-up: ramp the p-state clock while the first DMAs land.
        # warm tile is memset by Pool (no DMA dependency) so the PE can start
        # within ~1us of kernel entry; each matmul streams 512 cols.
        warm = pers.tile([128, BT], dt.bfloat16, name="warm", tag="warm")
        nc.gpsimd.memset(warm[:], 0)
        # warm-up matmuls write the same bank the gate logits use later
        small_ps = psg.tile([128, BT], dt.float32, name="small_ps", tag="small_ps")
        for i in range(NWARM):
            nc.tensor.matmul(small_ps[:], warm[:, 0:128], warm[:],
                             start=True, stop=True, skip_group_check=True)

        # ---- persistent loads ----
        # Two HWDGE queues: weights (cblob/wblob/bias) on the Act queue,
        # x tiles + stores on the SP queue, issued in consumption order.
        cb_sb = pers.tile([128, 400], dt.bfloat16, name="cb_sb", tag="cb_sb")
        nc.scalar.dma_start(cb_sb[:], cblob.ap())
        gwt_sb = cb_sb[:, 0:16]
        eye_sb = cb_sb[:, 16:144]
        sel_sb = cb_sb[0:4, 144:400]

        # bias is tiny and needed by layer-0 phase 3 (~15us in): load it
        # before the big wblob transfers so x-load bandwidth contention
        # can't push it past its first use (measured: a late bias stalls
        # the whole statically-scheduled ACT stream ~10us).
        bias_sb = pers.tile([128, L * NC_CH], dt.float32, name="bias_sb", tag="bias_sb")
        nc.scalar.dma_start(bias_sb[:], biasp.ap())

        x0b = []
        for t in range(NT):
            xt = pers.tile([128, NC_CH * BT], dt.bfloat16, name=f"x0b{t}", tag=f"x0b{t}")
            nc.sync.dma_start(xt[:], xbt.ap()[t])
            x0b.append(xt)

        wl_sb = []
        vg_sb = {}
        cbd_sb = {}
        ust_sb = {}
        for l in range(L):
            t_ = pers.tile([128, 2304], dt.bfloat16, name=f"wl{l}", tag=f"wl{l}")
            nc.scalar.dma_start(t_[:], wblob.ap()[l])
            wl_sb.append(t_)
            for g in range(NG):
                for c in range(NC_CH):
                    vg_sb[(l, g, c)] = t_[:, 128 * (4 * g + c):128 * (4 * g + c + 1)]
                cbd_sb[(l, g)] = t_[:, 1024 + 128 * g:1024 + 128 * (g + 1)]
                ust_sb[(l, g)] = t_[:, 1280 + 512 * g:1280 + 512 * (g + 1)]

        # f32 x (final-layer epilogue only, ~70us in): the tiles are declared
        # here but their DMAs are gated on layer-0 phase-3 progress (see
        # phase3) — descriptors are FIFO per DMA engine, so issuing these
        # 4MB up front starves the weight/x loads the early layers need.
        x0f = []
        for t in range(NT):
            xt = pers.tile([128, NC_CH * BT], dt.float32, name=f"x0f{t}", tag=f"x0f{t}")
            x0f.append(xt)

        # Single-wait discipline: DVE/ACT/Pool instructions carry at most one
        # sem wait in this ISA. Touch DMA-loaded regions just before the
        # engine first reads them so each compute op has one un-observed
        # dependency and no toucher stalls on a DMA that lands late.
        scratch = pers.tile([4, 16], dt.float32, name="scratch", tag="scratch")

        Exp = mybir.ActivationFunctionType.Exp
        Tanh = mybir.ActivationFunctionType.Tanh
        Ident = mybir.ActivationFunctionType.Identity
        add = mybir.AluOpType.add
        mult = mybir.AluOpType.mult

        A_sb = {}     # (t, g) -> SBUF bf16 A pair tile [128, 2*BT] of current layer
        xl_sb = {}    # (t, g) -> SBUF bf16 xl pair tile [128, 2*BT]

        def xsrc(l, c, t):
            if l == 0:
                return x0b[t][:, bass.ts(c, BT)]
            return xl_sb[(t, c // 2)][:, bass.ts(c % 2, BT)]

        for l in range(L):
            A_new = {}
            xl_new = {}

            # ---- phase 1: gate logits + softmax -> g_all[t] (bf16 SBUF) ----
            g_all = {}
            for t in range(NT):
                # logits for tile t at partitions 32*(t%3): psum base
                # partition must be 0/32/64; tile 3 reuses tile 0's rows
                # (its exp read is long done by then)
                pb = 32 * (t % 3)
                lg = small_ps[pb:pb + 4, :]
                for c in range(NC_CH):
                    nc.tensor.matmul(
                        lg, gwt_sb[:, bass.ts(c, 4)], xsrc(l, c, t),
                        start=(c == 0), stop=(c == NC_CH - 1),
                    )
                # Softmax normalization with batch on partitions: 32x32
                # stream-transpose puts the 4 expert weights of 32 batch cols
                # in each partition row; reduce+reciprocal then run on 128
                # lanes instead of 4 (a [4,BT] reciprocal costs ~3.3us).
                w_sb = sm.tile([32, BT], dt.bfloat16, name=f"w{l}{t}", tag="w")
                nc.scalar.activation(w_sb[0:4, :], lg, Exp)
                wt = sm.tile([32, BT], dt.bfloat16, name=f"wt{l}{t}", tag="wt")
                nc.vector.transpose(wt[:], w_sb[:])
                wt3 = wt[:].rearrange("p (b q) -> p b q", q=32)[:, :, 0:4]
                s32 = sm.tile([32, 16], dt.float32, name=f"s32{l}{t}", tag="s32")
                nc.vector.tensor_reduce(s32[:], wt3, axis=mybir.AxisListType.X,
                                        op=mybir.AluOpType.add)
                rs32 = sm.tile([32, 16], dt.float32, name=f"rs32{l}{t}", tag="rs32")
                nc.vector.reciprocal(rs32[:], s32[:])
                gt = sm.tile([32, BT], dt.bfloat16, name=f"gt{l}{t}", tag="gt")
                gt3 = gt[:].rearrange("p (b q) -> p b q", q=32)[:, :, 0:4]
                rs3 = rs32[:].rearrange("p b -> p b ()").broadcast_to([32, 16, 4])
                # normalize on Pool: keeps both-SBUF-operand multiplies off
                # DVE's shared port (Pool would block them for a whole op)
                nc.gpsimd.tensor_tensor(gt3, wt3, rs3, op=mybir.AluOpType.mult)
                g_sb = sm.tile([32, BT], dt.bfloat16, name=f"g{l}{t}", tag="g")
                nc.vector.transpose(g_sb[:], gt[:])
                g_all[t] = g_sb

            # ---- phase 2: experts (V -> tanh -> C -> tanh -> gate) ----
            gc_all = {}

            def emit_v(t):
                # both expert pairs share wide [128,1024] psum tiles so each
                # tanh is one ACT op instead of two
                vx = psex.tile([128, 2 * BT], dt.float32, name=f"vx{l}{t}", tag="ex")
                for g in range(NG):
                    for c in range(NC_CH):
                        nc.tensor.matmul(vx[:, bass.ts(g, BT)], vg_sb[(l, g, c)],
                                         xsrc(l, c, t),
                                         start=(c == 0), stop=(c == NC_CH - 1))
                tv = work.tile([128, 2 * BT], dt.bfloat16, name=f"tv{l}{t}", tag="tv")
                nc.scalar.activation(tv[:], vx[:], Tanh)
                return tv

            def emit_c(t, tv):
                cv = psex.tile([128, 2 * BT], dt.float32, name=f"cv{l}{t}", tag="ex")
                for g in range(NG):
                    nc.tensor.matmul(cv[:, bass.ts(g, BT)], cbd_sb[(l, g)],
                                     tv[:, bass.ts(g, BT)], start=True, stop=True)
                tcv = work.tile([128, 2 * BT], dt.bfloat16, name=f"tc{l}{t}", tag="tc")
                nc.scalar.activation(tcv[:], cv[:], Tanh)
                gc_sb = []
                for g in range(NG):
                    # gate broadcast just-in-time; gb has one bank, so probe
                    # (advancing DVE's PE clock) precedes each gating mul
                    p = psgb.tile([128, BT], dt.float32, name=f"gb{l}{t}{g}", tag="gb")
                    nc.tensor.matmul(p[:], sel_sb[:, bass.ts(g, 128)], g_all[t][0:4, :],
                                     start=True, stop=True)
                    nc.vector.tensor_copy(scratch[:, 8:12], p[0:4, 0:4])
                    gc = work.tile([128, BT], dt.bfloat16, name=f"gc{l}{t}{g}", tag="gc")
                    nc.vector.tensor_mul(gc[:], tcv[:, bass.ts(g, BT)], p[:])
                    gc_sb.append(gc)
                gc_all[t] = gc_sb

            # one-tile lookahead: V(t+1) is emitted before C(t) so the PE
            # stays busy across the tanh latency of tile t
            tv_pend = {}
            tv_pend[0] = emit_v(0)
            for t in range(NT):
                if t + 1 < NT:
                    tv_pend[t + 1] = emit_v(t + 1)
                emit_c(t, tv_pend.pop(t))

            # ---- phase 3: U matmuls + residual accumulate + epilogue ----
            def phase3(t):
                gc_sb = gc_all.pop(t)
                if l == L - 1:
                    nc.vector.tensor_copy(scratch[:, 0:4], x0f[t][0:4, 0:4])
                ap_pend = {}
                for g in range(NG):
                    an = apool.tile([128, 2 * BT], dt.bfloat16, name=f"A{l}{t}{g}", tag="A")
                    A_new[(t, g)] = an
                for c in range(NC_CH):
                    pa = psa.tile([128, BT], dt.float32, name=f"pa{l}{t}{c}", tag="pa")
                    nc.tensor.matmul(pa[:], ust_sb[(l, 0)][:, bass.ts(c, 128)],
                                     gc_sb[0][:], start=True, stop=False)
                    nc.tensor.matmul(pa[:], ust_sb[(l, 1)][:, bass.ts(c, 128)],
                                     gc_sb[1][:], start=False, stop=(l != 2))
                    if l == 2:
                        nc.tensor.matmul(pa[:], eye_sb,
                                         A_sb[(t, c // 2)][:, bass.ts(c % 2, BT)],
                                         start=False, stop=True)
                    bias_ap = bias_sb[:, l * NC_CH + c: l * NC_CH + c + 1]
                    if l == 0:
                        an = A_new[(t, c // 2)]
                        nc.scalar.activation(an[:, bass.ts(c % 2, BT)], pa[:],
                                             Ident, bias=bias_ap)
                    elif l == 1:
                        # A' = (mix + bias) + A_prev in one DVE op (psum in0 +
                        # single SBUF read keeps the shared port free)
                        an = A_new[(t, c // 2)]
                        nc.vector.scalar_tensor_tensor(
                            out=an[:, bass.ts(c % 2, BT)], in0=pa[:], scalar=bias_ap,
                            in1=A_sb[(t, c // 2)][:, bass.ts(c % 2, BT)],
                            op0=add, op1=add,
                        )
                    else:
                        # stores alternate across both HWDGE rings so the
                        # final 4MB drain isn't serialized on one issue queue
                        ot = opool.tile([128, BT], dt.float32, name=f"o{l}{t}{c}", tag="o")
                        nc.vector.scalar_tensor_tensor(
                            out=ot[:], in0=pa[:], scalar=bias_ap,
                            in1=x0f[t][:, bass.ts(c, BT)], op0=add, op1=mult,
                        )
                        eng = nc.sync if c % 2 == 0 else nc.scalar
                        eng.dma_start(outT.ap()[c, :, t * BT:(t + 1) * BT], ot[:])
                if l < L - 1:
                    # xl = A * x0 on the Pool engine (its own SBUF port;
                    # 2-SBUF-operand multiplies would contend on DVE)
                    for g in range(NG):
                        xn = xla.tile([128, 2 * BT], dt.bfloat16, name=f"xl{l}{t}{g}", tag="xl")
                        nc.gpsimd.tensor_mul(xn[:], A_new[(t, g)][:],
                                             x0b[t][:, bass.ts(g, 2 * BT)])
                        xl_new[(t, g)] = xn
                if l == 0:
                    # release the x0f[t] load now: a dummy write that waits on
                    # this tile's A (WAW with the dma) delays its descriptors
                    # past the startup load burst
                    nc.vector.tensor_copy(x0f[t][0:4, 0:4], A_new[(t, 0)][0:4, 0:4])
                    nc.sync.dma_start(x0f[t][:], xft.ap()[t])

            if l == 0:
                nc.scalar.copy(scratch[:, 4:8], bias_sb[0:4, 0:4])
            for t in range(NT):
                phase3(t)
            A_sb = A_new
            xl_sb = xl_new

    nc.compile()
    return nc


def _pack_inputs(input, U, V, C, bias, gate_w):
    """Host-side packing of full inputs into per-core DRAM tensor maps."""
    x = np.asarray(input, dtype=np.float32)
    U = np.asarray(U, dtype=np.float32)
    V = np.asarray(V, dtype=np.float32)
    C = np.asarray(C, dtype=np.float32)
    bias = np.asarray(bias, dtype=np.float32)
    gate_w = np.asarray(gate_w, dtype=np.float32)

    xT = np.ascontiguousarray(x.T)                      # [D, B]
    wblob = np.zeros((L, 128, 2304), dtype=BF)
    for l in range(L):
        for g in range(NG):
            pair = np.concatenate([V[l, 2 * g], V[l, 2 * g + 1]], axis=1)  # [512,128]
            pr = pair.reshape(NC_CH, 128, 128)
            for c in range(NC_CH):
                wblob[l, :, 128 * (4 * g + c):128 * (4 * g + c + 1)] = pr[c].astype(BF)
            cb = np.zeros((128, 128), dtype=np.float32)
            cb[0:64, 0:64] = C[l, 2 * g]
            cb[64:128, 64:128] = C[l, 2 * g + 1]
            wblob[l, :, 1024 + 128 * g:1024 + 128 * (g + 1)] = cb.astype(BF)
            wblob[l, :, 1280 + 512 * g:1280 + 512 * (g + 1)] = np.concatenate(
                [U[l, 2 * g].T, U[l, 2 * g + 1].T], axis=0).astype(BF)
    cblob = np.zeros((128, 400), dtype=BF)
    cblob[:, 0:16] = gate_w.reshape(E, NC_CH, 128).transpose(2, 1, 0).reshape(128, NC_CH * E).astype(BF)
    cblob[:, 16:144] = np.eye(128, dtype=BF)
    for g in range(NG):
        cblob[2 * g, 144 + 128 * g:144 + 128 * g + 64] = 1
        cblob[2 * g + 1, 144 + 128 * g + 64:144 + 128 * (g + 1)] = 1
    biasp = np.empty((128, L * NC_CH), dtype=np.float32)
    for l in range(L):
        for c in range(NC_CH):
            biasp[:, l * NC_CH + c] = bias[l, 128 * c: 128 * (c + 1)]
    biasp[:, 0:NC_CH] += 1.0    # layer-0 folds A=1 init

    shared = {
        "wblob": wblob,
        "cblob": cblob,
        "biasp": biasp,
    }
    in_maps = []
    for k in range(NCORES):
        xs = xT[:, k * BLOC:(k + 1) * BLOC]             # [512, 2048]
        # per batch tile, chunk-major free dim: [t, p, c*BT+j] = xs[c*128+p, t*BT+j]
        xtile = np.ascontiguousarray(
            xs.reshape(NC_CH, 128, NT, BT).transpose(2, 1, 0, 3).reshape(NT, 128, NC_CH * BT)
        )
        m = dict(shared)
        m["xbt"] = xtile.astype(BF)
        m["xft"] = xtile
        in_maps.append(m)
    return in_maps


def run(inputs, trace=False):
    if "nc" not in _cache:
        _cache["nc"] = _build()
    nc = _cache["nc"]
    in_maps = _pack_inputs(**inputs)
    res = run_bass_kernel_spmd(nc, in_maps, list(range(NCORES)), trace=trace)
    outs = []
    for k in range(NCORES):
        oT = np.asarray(res.results[k]["outT"], dtype=np.float32)  # [4,128,2048]
        outs.append(oT.reshape(D, BLOC).T)                         # [2048, 512]
    full = np.concatenate(outs, axis=0)                            # [B, D]
    return full, res


def kernel(**inputs):
    return run(inputs)[0]


# revision 23
# speedup vs baseline: 1.1477x; 1.0049x over previous
"""Trainium2 Bass kernel for nn_CrossNetworkMix (cross-network MoE mixer).

Reference computation (per layer, L=3):
    gate = softmax(xl @ gate_w.T)                 # [B, E]
    Vx   = tanh(einsum('edr,bd->ber', V[l], xl))  # [B, E, R]
    CVx  = tanh(einsum('esr,bes->ber', C[l], Vx)) # [B, E, R]
    UCVx = einsum('edr,ber->bed', U[l], CVx)      # [B, E, D]
    xl   = einsum('be,bed->bd', gate, x0[:,None,:]*(UCVx + bias[l])) + xl

Key algebraic identity used: sum_e gate[b,e] = 1, so
    xl_{l+1} = x0 dot (mix_l + bias_l) + xl_l,  mix_l = sum_e g_e * (U_e @ tanh(...))
and by induction xl_l = x0 * A_l with A_l = 1 + sum_{j<l} (mix_j + bias_j).
The kernel carries the accumulator A (residual never materialized in f32).

Device layout: activations transposed [feature, batch]; batch B=16384 sharded
8 ways (2048 cols/core), processed in 4 column tiles of 512 (PSUM free dim).
Experts packed in pairs into 128-wide matmuls. All matmuls bf16 with f32 PSUM
accumulation; softmax reduction/broadcast across the tiny E=4 partition dim is
done with small matmuls (ones / selector / identity stationary operands).

Scheduling notes (all measured on HW traces):
 - The PE clock ramps 0.65 -> 1.2 -> 2.4 GHz and needs ~3us of continuous
   work for full speed, so dummy warm-up matmuls run while the first DMAs
   land.
 - x is loaded in per-batch-tile pieces so tile 0's matmuls start ~1.5us in.
 - Elementwise xl = A*x0 products run on the otherwise-idle Pool (GpSimd)
   engine.  DVE ops are arranged to have at most one SBUF read operand so
   they never arbitrate for the DVE/Pool shared SBUF port (exclusive lock).
 - Weight loads go out on the Act HWDGE queue, x loads + output stores on
   the SP queue, so neither queue's ~0.6us per-DMA issue time serializes.
"""

from contextlib import ExitStack

import numpy as np
import ml_dtypes

import concourse.bass as bass
import concourse.tile as tile
from concourse import bacc
from concourse import mybir
from concourse.bass_utils import run_bass_kernel_spmd

BF = ml_dtypes.bfloat16

B, D, L, R, E = 16384, 512, 3, 64, 4
NCORES = 8
BLOC = B // NCORES          # batch columns per core
NT = BLOC // 512            # batch tiles per core (512 cols each)
BT = 512                    # batch tile width
NC_CH = D // 128            # feature chunks (4)
NG = E // 2                 # expert pair groups (2)
NWARM = 8                   # PE warm-up matmuls (p-state ramp + DMA shadow)

_cache = {}


def _build():
    nc = bacc.Bacc("TRN2", target_bir_lowering=False, debug=False)
    dt = mybir.dt

    # ---- DRAM I/O ----
    # x per batch tile, chunk-major in the free dim: [t][p][c*BT+j] = x[c*128+p, t*BT+j]
    xbt = nc.dram_tensor("xbt", [NT, 128, NC_CH * BT], dt.bfloat16, kind="ExternalInput")
    xft = nc.dram_tensor("xft", [NT, 128, NC_CH * BT], dt.float32, kind="ExternalInput")
    # weights packed per layer into one blob: vg (8x128 cols) | cbd (2x128) |
    # ust (2x512) = [128, 2304]; consts into one bf16 blob = [128, 400]
    wblob = nc.dram_tensor("wblob", [L, 128, 2304], dt.bfloat16, kind="ExternalInput")
    cblob = nc.dram_tensor("cblob", [128, 400], dt.bfloat16, kind="ExternalInput")
    biasp = nc.dram_tensor("biasp", [128, L * NC_CH], dt.float32, kind="ExternalInput")
    outT = nc.dram_tensor("outT", [NC_CH, 128, BLOC], dt.float32, kind="ExternalOutput")

    with tile.TileContext(nc) as tc, ExitStack() as ctx:
        pers = ctx.enter_context(tc.tile_pool(name="pers", bufs=1))
        psg = ctx.enter_context(tc.tile_pool(name="psg", bufs=1, space="PSUM"))
        psgb = ctx.enter_context(tc.tile_pool(name="psgb", bufs=1, space="PSUM"))
        psex = ctx.enter_context(tc.tile_pool(name="psex", bufs=2, space="PSUM"))
        psa = ctx.enter_context(tc.tile_pool(name="psa", bufs=2, space="PSUM"))
        sm = ctx.enter_context(tc.tile_pool(name="sm", bufs=4))     # small gate tiles
        work = ctx.enter_context(tc.tile_pool(name="work", bufs=3))  # tanh/gc tiles
        xla = ctx.enter_context(tc.tile_pool(name="xla", bufs=NT * NG + 2))   # xl pair tiles
        apool = ctx.enter_context(tc.tile_pool(name="apool", bufs=NT * NG + 2))  # A pair tiles
        opool = ctx.enter_context(tc.tile_pool(name="opool", bufs=NT * NC_CH))

        # ---- PE warm-up: ramp the p-state clock while the first DMAs land.
        # warm tile is memset by Pool (no DMA dependency) so the PE can start
        # within ~1us of kernel entry; each matmul streams 512 cols.
        warm = pers.tile([128, BT], dt.bfloat16, name="warm", tag="warm")
        nc.gpsimd.memset(warm[:], 0)
        # warm-up matmuls write the same bank the gate logits use later
        small_ps = psg.tile([128, BT], dt.float32, name="small_ps", tag="small_ps")
        for i in range(NWARM):
            nc.tensor.matmul(small_ps[:], warm[:, 0:128], warm[:],
                             start=True, stop=True, skip_group_check=True)

        # ---- persistent loads ----
        # Two HWDGE queues: weights (cblob/wblob/bias) on the Act queue,
        # x tiles + stores on the SP queue, issued in consumption order.
        cb_sb = pers.tile([128, 400], dt.bfloat16, name="cb_sb", tag="cb_sb")
        nc.scalar.dma_start(cb_sb[:], cblob.ap())
        gwt_sb = cb_sb[:, 0:16]
        eye_sb = cb_sb[:, 16:144]
        sel_sb = cb_sb[0:4, 144:400]

        # bias is tiny and needed by layer-0 phase 3 (~15us in): load it
        # before the big wblob transfers so x-load bandwidth contention
        # can't push it past its first use (measured: a late bias stalls
        # the whole statically-scheduled ACT stream ~10us).
        bias_sb = pers.tile([128, L * NC_CH], dt.float32, name="bias_sb", tag="bias_sb")
        nc.scalar.dma_start(bias_sb[:], biasp.ap())

        # layer-0 weights FIRST on the SP ring (descriptors are FIFO per DMA
        # engine: issued behind 2MB of x they'd land ~8us, stalling V(0))
        wl_sb = []
        vg_sb = {}
        cbd_sb = {}
        ust_sb = {}
        for l in range(L):
            t_ = pers.tile([128, 2304], dt.bfloat16, name=f"wl{l}", tag=f"wl{l}")
            wl_sb.append(t_)
            for g in range(NG):
                for c in range(NC_CH):
                    vg_sb[(l, g, c)] = t_[:, 128 * (4 * g + c):128 * (4 * g + c + 1)]
                cbd_sb[(l, g)] = t_[:, 1024 + 128 * g:1024 + 128 * (g + 1)]
                ust_sb[(l, g)] = t_[:, 1280 + 512 * g:1280 + 512 * (g + 1)]
        nc.sync.dma_start(wl_sb[0][:], wblob.ap()[0])

        x0b = []
        for t in range(NT):
            xt = pers.tile([128, NC_CH * BT], dt.bfloat16, name=f"x0b{t}", tag=f"x0b{t}")
            nc.sync.dma_start(xt[:], xbt.ap()[t])
            x0b.append(xt)
        nc.scalar.dma_start(wl_sb[1][:], wblob.ap()[1])
        nc.scalar.dma_start(wl_sb[2][:], wblob.ap()[2])

        # f32 x (final-layer epilogue only, ~70us in): the tiles are declared
        # here but their DMAs are gated on layer-0 phase-3 progress (see
        # phase3) — descriptors are FIFO per DMA engine, so issuing these
        # 4MB up front starves the weight/x loads the early layers need.
        x0f = []
        for t in range(NT):
            xt = pers.tile([128, NC_CH * BT], dt.float32, name=f"x0f{t}", tag=f"x0f{t}")
            x0f.append(xt)

        # Single-wait discipline: DVE/ACT/Pool instructions carry at most one
        # sem wait in this ISA. Touch DMA-loaded regions just before the
        # engine first reads them so each compute op has one un-observed
        # dependency and no toucher stalls on a DMA that lands late.
        scratch = pers.tile([4, 16], dt.float32, name="scratch", tag="scratch")

        Exp = mybir.ActivationFunctionType.Exp
        Tanh = mybir.ActivationFunctionType.Tanh
        Ident = mybir.ActivationFunctionType.Identity
        add = mybir.AluOpType.add
        mult = mybir.AluOpType.mult

        A_sb = {}     # (t, g) -> SBUF bf16 A pair tile [128, 2*BT] of current layer
        xl_sb = {}    # (t, g) -> SBUF bf16 xl pair tile [128, 2*BT]

        def xsrc(l, c, t):
            if l == 0:
                return x0b[t][:, bass.ts(c, BT)]
            return xl_sb[(t, c // 2)][:, bass.ts(c % 2, BT)]

        for l in range(L):
            A_new = {}
            xl_new = {}

            # ---- phase 1: gate logits + softmax -> g_all[t] (bf16 SBUF) ----
            g_all = {}
            ctx1 = nc.named_scope(f"L{l}ph1"); ctx1.__enter__()
            for t in range(NT):
                # logits for tile t at partitions 32*(t%3): psum base
                # partition must be 0/32/64; tile 3 reuses tile 0's rows
                # (its exp read is long done by then)
                pb = 32 * (t % 3)
                lg = small_ps[pb:pb + 4, :]
                for c in range(NC_CH):
                    nc.tensor.matmul(
                        lg, gwt_sb[:, bass.ts(c, 4)], xsrc(l, c, t),
                        start=(c == 0), stop=(c == NC_CH - 1),
                    )
                # Softmax normalization with batch on partitions: 32x32
                # stream-transpose puts the 4 expert weights of 32 batch cols
                # in each partition row; reduce+reciprocal then run on 128
                # lanes instead of 4 (a [4,BT] reciprocal costs ~3.3us).
                w_sb = sm.tile([32, BT], dt.bfloat16, name=f"w{l}{t}", tag="w")
                nc.scalar.activation(w_sb[0:4, :], lg, Exp)
                wt = sm.tile([32, BT], dt.bfloat16, name=f"wt{l}{t}", tag="wt")
                nc.vector.transpose(wt[:], w_sb[:])
                wt3 = wt[:].rearrange("p (b q) -> p b q", q=32)[:, :, 0:4]
                s32 = sm.tile([32, 16], dt.float32, name=f"s32{l}{t}", tag="s32")
                nc.vector.tensor_reduce(s32[:], wt3, axis=mybir.AxisListType.X,
                                        op=mybir.AluOpType.add)
                rs32 = sm.tile([32, 16], dt.float32, name=f"rs32{l}{t}", tag="rs32")
                nc.vector.reciprocal(rs32[:], s32[:])
                gt = sm.tile([32, BT], dt.bfloat16, name=f"gt{l}{t}", tag="gt")
                gt3 = gt[:].rearrange("p (b q) -> p b q", q=32)[:, :, 0:4]
                rs3 = rs32[:].rearrange("p b -> p b ()").broadcast_to([32, 16, 4])
                # normalize on Pool: keeps both-SBUF-operand multiplies off
                # DVE's shared port (Pool would block them for a whole op)
                nc.gpsimd.tensor_tensor(gt3, wt3, rs3, op=mybir.AluOpType.mult)
                g_sb = sm.tile([32, BT], dt.bfloat16, name=f"g{l}{t}", tag="g")
                nc.vector.transpose(g_sb[:], gt[:])
                g_all[t] = g_sb

            # ---- phase 2: experts (V -> tanh -> C -> tanh -> gate) ----
            gc_all = {}

            def emit_v(t):
                # both expert pairs share wide [128,1024] psum tiles so each
                # tanh is one ACT op instead of two
                vx = psex.tile([128, 2 * BT], dt.float32, name=f"vx{l}{t}", tag="ex")
                for g in range(NG):
                    for c in range(NC_CH):
                        nc.tensor.matmul(vx[:, bass.ts(g, BT)], vg_sb[(l, g, c)],
                                         xsrc(l, c, t),
                                         start=(c == 0), stop=(c == NC_CH - 1))
                tv = work.tile([128, 2 * BT], dt.bfloat16, name=f"tv{l}{t}", tag="tv")
                nc.scalar.activation(tv[:], vx[:], Tanh)
                return tv

            def emit_c(t, tv):
                cv = psex.tile([128, 2 * BT], dt.float32, name=f"cv{l}{t}", tag="ex")
                for g in range(NG):
                    nc.tensor.matmul(cv[:, bass.ts(g, BT)], cbd_sb[(l, g)],
                                     tv[:, bass.ts(g, BT)], start=True, stop=True)
                tcv = work.tile([128, 2 * BT], dt.bfloat16, name=f"tc{l}{t}", tag="tc")
                nc.scalar.activation(tcv[:], cv[:], Tanh)
                gc_sb = []
                for g in range(NG):
                    # gate broadcast just-in-time; gb has one bank, so probe
                    # (advancing DVE's PE clock) precedes each gating mul
                    p = psgb.tile([128, BT], dt.float32, name=f"gb{l}{t}{g}", tag="gb")
                    nc.tensor.matmul(p[:], sel_sb[:, bass.ts(g, 128)], g_all[t][0:4, :],
                                     start=True, stop=True)
                    nc.vector.tensor_copy(scratch[:, 8:12], p[0:4, 0:4])
                    gc = work.tile([128, BT], dt.bfloat16, name=f"gc{l}{t}{g}", tag="gc")
                    nc.vector.tensor_mul(gc[:], tcv[:, bass.ts(g, BT)], p[:])
                    gc_sb.append(gc)
                gc_all[t] = gc_sb

            ctx1.__exit__(None, None, None)
            # one-tile lookahead: V(t+1) is emitted before C(t) so the PE
            # stays busy across the tanh latency of tile t
            ctx2 = nc.named_scope(f"L{l}ph2"); ctx2.__enter__()
            tv_pend = {}
            tv_pend[0] = emit_v(0)
            for t in range(NT):
                if t + 1 < NT:
                    tv_pend[t + 1] = emit_v(t + 1)
                emit_c(t, tv_pend.pop(t))
            ctx2.__exit__(None, None, None)

            # ---- phase 3: U matmuls + residual accumulate + epilogue ----
            def phase3(t):
                gc_sb = gc_all.pop(t)
                if l == L - 1:
                    nc.vector.tensor_copy(scratch[:, 0:4], x0f[t][0:4, 0:4])
                ap_pend = {}
                for g in range(NG):
                    an = apool.tile([128, 2 * BT], dt.bfloat16, name=f"A{l}{t}{g}", tag="A")
                    A_new[(t, g)] = an
                for c in range(NC_CH):
                    pa = psa.tile([128, BT], dt.float32, name=f"pa{l}{t}{c}", tag="pa")
                    nc.tensor.matmul(pa[:], ust_sb[(l, 0)][:, bass.ts(c, 128)],
                                     gc_sb[0][:], start=True, stop=False)
                    nc.tensor.matmul(pa[:], ust_sb[(l, 1)][:, bass.ts(c, 128)],
                                     gc_sb[1][:], start=False, stop=(l != 2))
                    if l == 2:
                        nc.tensor.matmul(pa[:], eye_sb,
                                         A_sb[(t, c // 2)][:, bass.ts(c % 2, BT)],
                                         start=False, stop=True)
                    bias_ap = bias_sb[:, l * NC_CH + c: l * NC_CH + c + 1]
                    if l == 0:
                        an = A_new[(t, c // 2)]
                        nc.scalar.activation(an[:, bass.ts(c % 2, BT)], pa[:],
                                             Ident, bias=bias_ap)
                    elif l == 1:
                        # A' = (mix + bias) + A_prev in one DVE op (psum in0 +
                        # single SBUF read keeps the shared port free)
                        an = A_new[(t, c // 2)]
                        nc.vector.scalar_tensor_tensor(
                            out=an[:, bass.ts(c % 2, BT)], in0=pa[:], scalar=bias_ap,
                            in1=A_sb[(t, c // 2)][:, bass.ts(c % 2, BT)],
                            op0=add, op1=add,
                        )
                    else:
                        # stores alternate across both HWDGE rings so the
                        # final 4MB drain isn't serialized on one issue queue
                        ot = opool.tile([128, BT], dt.float32, name=f"o{l}{t}{c}", tag="o")
                        nc.vector.scalar_tensor_tensor(
                            out=ot[:], in0=pa[:], scalar=bias_ap,
                            in1=x0f[t][:, bass.ts(c, BT)], op0=add, op1=mult,
                        )
                        eng = nc.sync if c % 2 == 0 else nc.scalar
                        eng.dma_start(outT.ap()[c, :, t * BT:(t + 1) * BT], ot[:])
                if l < L - 1:
                    # xl = A * x0 on the Pool engine (its own SBUF port;
                    # 2-SBUF-operand multiplies would contend on DVE)
                    for g in range(NG):
                        xn = xla.tile([128, 2 * BT], dt.bfloat16, name=f"xl{l}{t}{g}", tag="xl")
                        nc.gpsimd.tensor_mul(xn[:], A_new[(t, g)][:],
                                             x0b[t][:, bass.ts(g, 2 * BT)])
                        xl_new[(t, g)] = xn
                if l == 0:
                    # release the x0f[t] load now: a dummy write that waits on
                    # this tile's A (WAW with the dma) delays its descriptors
                    # past the startup load burst
                    nc.vector.tensor_copy(x0f[t][0:4, 0:4], A_new[(t, 0)][0:4, 0:4])
                    nc.sync.dma_start(x0f[t][:], xft.ap()[t])

            if l == 0:
                nc.scalar.copy(scratch[:, 4:8], bias_sb[0:4, 0:4])
            ctx3 = nc.named_scope(f"L{l}ph3"); ctx3.__enter__()
            for t in range(NT):
                phase3(t)
            ctx3.__exit__(None, None, None)
            A_sb = A_new
            xl_sb = xl_new

    nc.compile()
    return nc


def _pack_inputs(input, U, V, C, bias, gate_w):
    """Host-side packing of full inputs into per-core DRAM tensor maps."""
    x = np.asarray(input, dtype=np.float32)
    U = np.asarray(U, dtype=np.float32)
    V = np.asarray(V, dtype=np.float32)
    C = np.asarray(C, dtype=np.float32)
    bias = np.asarray(bias, dtype=np.float32)
    gate_w = np.asarray(gate_w, dtype=np.float32)

    xT = np.ascontiguousarray(x.T)                      # [D, B]
    wblob = np.zeros((L, 128, 2304), dtype=BF)
    for l in range(L):
        for g in range(NG):
            pair = np.concatenate([V[l, 2 * g], V[l, 2 * g + 1]], axis=1)  # [512,128]
            pr = pair.reshape(NC_CH, 128, 128)
            for c in range(NC_CH):
                wblob[l, :, 128 * (4 * g + c):128 * (4 * g + c + 1)] = pr[c].astype(BF)
            cb = np.zeros((128, 128), dtype=np.float32)
            cb[0:64, 0:64] = C[l, 2 * g]
            cb[64:128, 64:128] = C[l, 2 * g + 1]
            wblob[l, :, 1024 + 128 * g:1024 + 128 * (g + 1)] = cb.astype(BF)
            wblob[l, :, 1280 + 512 * g:1280 + 512 * (g + 1)] = np.concatenate(
                [U[l, 2 * g].T, U[l, 2 * g + 1].T], axis=0).astype(BF)
    cblob = np.zeros((128, 400), dtype=BF)
    cblob[:, 0:16] = gate_w.reshape(E, NC_CH, 128).transpose(2, 1, 0).reshape(128, NC_CH * E).astype(BF)
    cblob[:, 16:144] = np.eye(128, dtype=BF)
    for g in range(NG):
        cblob[2 * g, 144 + 128 * g:144 + 128 * g + 64] = 1
        cblob[2 * g + 1, 144 + 128 * g + 64:144 + 128 * (g + 1)] = 1
    biasp = np.empty((128, L * NC_CH), dtype=np.float32)
    for l in range(L):
        for c in range(NC_CH):
            biasp[:, l * NC_CH + c] = bias[l, 128 * c: 128 * (c + 1)]
    biasp[:, 0:NC_CH] += 1.0    # layer-0 folds A=1 init

    shared = {
        "wblob": wblob,
        "cblob": cblob,
        "biasp": biasp,
    }
    in_maps = []
    for k in range(NCORES):
        xs = xT[:, k * BLOC:(k + 1) * BLOC]             # [512, 2048]
        # per batch tile, chunk-major free dim: [t, p, c*BT+j] = xs[c*128+p, t*BT+j]
        xtile = np.ascontiguousarray(
            xs.reshape(NC_CH, 128, NT, BT).transpose(2, 1, 0, 3).reshape(NT, 128, NC_CH * BT)
        )
        m = dict(shared)
        m["xbt"] = xtile.astype(BF)
        m["xft"] = xtile
        in_maps.append(m)
    return in_maps


def run(inputs, trace=False):
    if "nc" not in _cache:
        _cache["nc"] = _build()
    nc = _cache["nc"]
    in_maps = _pack_inputs(**inputs)
    res = run_bass_kernel_spmd(nc, in_maps, list(range(NCORES)), trace=trace)
    outs = []
    for k in range(NCORES):
        oT = np.asarray(res.results[k]["outT"], dtype=np.float32)  # [4,128,2048]
        outs.append(oT.reshape(D, BLOC).T)                         # [2048, 512]
    full = np.concatenate(outs, axis=0)                            # [B, D]
    return full, res


def kernel(**inputs):
    return run(inputs)[0]
